# revision 2
# baseline (speedup 1.0000x reference)
"""3-layer GCN (gcn_norm + 3x gcn_conv + log_softmax) on 8 TRN2 NeuronCores.

Strategy (dst-sharded, graph-parallel):
  - Nodes split contiguously across 8 cores (12500 each, padded to
    12544 = 128*98 rows). The kernel works in a permuted "table row"
    space; the host un-permutes at the end.
  - GCN algebra refactored so no per-edge norm is needed:
        conv(h) = D^-1/2 (A+I) D^-1/2 (h W) + b
    with g = (h W) * dinv:  out[n] = dinv[n] * (g[n] + sum_{e:dst=n} g[src_e]) + b
    so each conv is: build table g (pre-scaled rows), AllGather, gather+sum
    rows per destination, scale by dinv, add bias.
  - gather+sum: bulk dma_gather (SWDGE custom op) of table rows into SBUF
    slot tiles, unit-stride DVE plane adds, then dma_scatter_add into a
    DRAM accumulator to reconcile per-window node orders (int16 gather
    indices only span a 32768-row window -> edges split into 4 source
    windows, each with its own degree-sorted block structure so slot
    padding stays small).
  - The tiny weight matmuls (13->64->64->10) run per 128-node block on
    the TensorEngine (transpose-via-identity + matmul).

kernel(**inputs) takes FULL unsharded inputs, returns FULL [100000, 10]
float32 log-softmax output.
"""
import os
import sys

import numpy as np

if "/opt/trn_rl_repo" not in sys.path:
    sys.path.insert(0, "/opt/trn_rl_repo")

N_NODES = 100000
N_EDGES = 1600000
N_CORES = 8
P = 128
NB = 98                      # node blocks per core
NLOC = P * NB                # 12544 padded nodes per core
NPC = N_NODES // N_CORES     # 12500 real nodes per core
NPAD = N_CORES * NLOC        # 100352 table rows
WIN = 32768                  # int16 index window (rows)
N_WIN = 4
W_BASES = [0, WIN, 2 * WIN, 3 * WIN]
W_SIZES = [WIN, WIN, WIN, NPAD - 3 * WIN]
TF = 64                      # table row stride in f32 (256B)
F1 = 16                      # conv1/conv3 gathered row width (f32)
GCHUNK = 8                   # slot columns per dma_gather call (keep SWDGE
                             # descriptor bursts small: 1024 descs/call)
SCHUNK = 24                  # blocks per dma_scatter_add call


# --------------------------------------------------------------------------
# host-side graph preprocessing (integer/layout work only)
# --------------------------------------------------------------------------

def _prep(edge_index):
    src = np.asarray(edge_index[0], dtype=np.int64)
    dst = np.asarray(edge_index[1], dtype=np.int64)

    deg_in = np.bincount(dst, minlength=N_NODES)
    core_of = np.arange(N_NODES) // NPC

    # canonical within-core order: total degree descending
    rank = np.empty(N_NODES, dtype=np.int64)
    for c in range(N_CORES):
        ids = np.arange(c * NPC, (c + 1) * NPC)
        o = np.argsort(-deg_in[ids], kind="stable")
        rank[ids[o]] = np.arange(NPC)
    p_of = rank % P
    b_of = rank // P
    r_loc = p_of * NB + b_of                 # canonical local row
    g_row = core_of * NLOC + r_loc           # global table row

    # message entries = edges + self loops; window = src row // WIN
    all_dst = np.concatenate([dst, np.arange(N_NODES)])
    all_gsrc = np.concatenate([g_row[src], g_row])
    all_w = all_gsrc // WIN
    all_core = all_dst // NPC

    # per (node, window) counts
    wcnt = np.bincount(all_dst * N_WIN + all_w,
                       minlength=N_NODES * N_WIN).reshape(N_NODES, N_WIN)

    # per (core, window): window-degree-sorted ranks
    wrank = np.empty((N_NODES, N_WIN), dtype=np.int64)
    wdeg_sorted = np.zeros((N_CORES, N_WIN, NLOC), dtype=np.int64)
    wnode_rloc = np.zeros((N_CORES, N_WIN, NLOC), dtype=np.int64)
    dr = np.arange(NPC, NLOC)
    dummy_rloc = (dr % P) * NB + dr // P
    for c in range(N_CORES):
        ids = np.arange(c * NPC, (c + 1) * NPC)
        for w in range(N_WIN):
            o = np.argsort(-wcnt[ids, w], kind="stable")
            wrank[ids[o], w] = np.arange(NPC)
            wdeg_sorted[c, w, :NPC] = wcnt[ids[o], w]
            wnode_rloc[c, w, :NPC] = r_loc[ids[o]]
            wnode_rloc[c, w, NPC:] = dummy_rloc

    # shared slot structure (cross-core max; non-increasing in b)
    S = [wdeg_sorted[:, w, ::P].max(axis=0).astype(np.int64)
         for w in range(N_WIN)]
    B = []
    for w in range(N_WIN):
        s0 = int(S[w][0])
        B.append([int((S[w] > k).sum()) for k in range(s0)])
    SW = [int(S[w].sum()) for w in range(N_WIN)]
    plane_off = [np.concatenate([[0], np.cumsum(B[w])]).astype(np.int64)
                 for w in range(N_WIN)]

    # guaranteed-zero padding row per window (a dummy node; dinv = 0)
    all_dummies = (np.arange(N_CORES)[:, None] * NLOC +
                   dummy_rloc[None, :]).ravel()
    zero_row = []
    for w in range(N_WIN):
        base, size = W_BASES[w], W_SIZES[w]
        cand = all_dummies[(all_dummies >= base) & (all_dummies < base + size)]
        assert len(cand) > 0, f"no zero row in window {w}"
        zero_row.append(int(cand[0]))

    # fill gather index arrays (slot col of k-th entry = plane_off[k] + b)
    idx_lin = [np.full((N_CORES, SW[w] * P), zero_row[w] - W_BASES[w],
                       dtype=np.int64) for w in range(N_WIN)]
    ekey = (all_core * N_WIN + all_w) * NLOC + wrank[all_dst, all_w]
    order = np.argsort(ekey, kind="stable")
    se_key = ekey[order]
    se_gsrc = all_gsrc[order]
    is_start = np.ones(len(se_key), dtype=bool)
    is_start[1:] = se_key[1:] != se_key[:-1]
    grp_start_pos = np.flatnonzero(is_start)
    grp_id = np.cumsum(is_start) - 1
    k_within = np.arange(len(se_key)) - grp_start_pos[grp_id]

    se_c = se_key // (N_WIN * NLOC)
    se_w = (se_key // NLOC) % N_WIN
    se_r = se_key % NLOC
    se_p = se_r % P
    se_b = se_r // P
    for w in range(N_WIN):
        m = se_w == w
        col = plane_off[w][k_within[m]] + se_b[m]
        idx_lin[w][se_c[m], col * P + se_p[m]] = se_gsrc[m] - W_BASES[w]
        assert idx_lin[w].min() >= 0 and idx_lin[w].max() < W_SIZES[w]

    def wrap16(lin2d):
        # [cores, L] -> [cores, 16, L//16] with linear[i] at [:, i%16, i//16],
        # then replicate the 16-row pattern across 128 partitions.
        n = lin2d.shape[1]
        a = lin2d.reshape(N_CORES, n // 16, 16).transpose(0, 2, 1)
        a = np.ascontiguousarray(a).astype(np.int16)
        return np.tile(a, (1, 8, 1))  # [cores, 128, n//16]

    idx_cat = np.concatenate([wrap16(idx_lin[w]) for w in range(N_WIN)], axis=2)
    idx_woff = np.concatenate(
        [[0], np.cumsum([SW[w] * 8 for w in range(N_WIN)])]).astype(int)

    # scatter indices: token i (= w-rank) -> canonical local row
    B0 = [B[w][0] for w in range(N_WIN)]
    sidx_cat = np.concatenate(
        [wrap16(wnode_rloc[:, w, :B0[w] * P]) for w in range(N_WIN)], axis=2)
    sidx_woff = np.concatenate(
        [[0], np.cumsum([B0[w] * 8 for w in range(N_WIN)])]).astype(int)

    meta = dict(S=[s.tolist() for s in S], B=B, SW=SW, B0=B0,
                plane_off=[p.tolist() for p in plane_off],
                idx_woff=idx_woff.tolist(), sidx_woff=sidx_woff.tolist())
    perm = dict(core_of=core_of, r_loc=r_loc, deg_in=deg_in,
                p_of=p_of, b_of=b_of)
    return meta, perm, idx_cat, sidx_cat


def _pack_inputs(x, W1, b1, W2, b2, W3, b3, perm, idx_cat, sidx_cat):
    core_of, r_loc = perm["core_of"], perm["r_loc"]
    p_of, b_of, deg_in = perm["p_of"], perm["b_of"], perm["deg_in"]

    x_sb = np.zeros((N_CORES, P, NB * F1), dtype=np.float32)
    for j in range(13):
        x_sb[core_of, p_of, b_of * F1 + j] = x[:, j]
    deg_f = np.zeros((N_CORES, P, NB), dtype=np.float32)
    deg_f[core_of, p_of, b_of] = (deg_in + 1).astype(np.float32)

    wt = np.zeros((64, 144), dtype=np.float32)
    wt[:13, 0:64] = W1
    wt[:, 64:128] = W2
    wt[:, 128:138] = W3
    bt = np.zeros((P, 144), dtype=np.float32)
    bt[:, 0:64] = b1[None, :]
    bt[:, 64:128] = b2[None, :]
    bt[:, 128:138] = b3[None, :]
    bt[:, 138:144] = -1e30  # pad classes for batched log_softmax

    return [{
        "x_sb": x_sb[c], "deg": deg_f[c],
        "gidx": idx_cat[c], "sidx": sidx_cat[c],
        "wt": wt, "bt": bt,
    } for c in range(N_CORES)]


# --------------------------------------------------------------------------
# raw dma_gather emitter: bass.BassGpSimd.dma_gather minus the
# transpose-only elem%256B restriction (the q7 ucode only requires the row
# *stride* to be a multiple of 256B for the non-transpose HBM path).
# --------------------------------------------------------------------------

def _dma_gather_raw(eng, out_ap, in_ap, idxs_ap, num_idxs, elem_size,
                    elem_step):
    import concourse.mybir as mybir
    from concourse import ap_utils
    from concourse.bass import MemorySpace

    assert idxs_ap.dtype == mybir.dt.int16
    assert in_ap.dtype == out_ap.dtype
    assert in_ap.space == MemorySpace.DRAM
    assert idxs_ap.space == MemorySpace.SBUF
    assert out_ap.space == MemorySpace.SBUF
    assert ap_utils.ap_is_contiguous(out_ap.ap[1:])
    assert ap_utils.ap_is_contiguous(idxs_ap.ap[1:])
    assert num_idxs % P == 0
    assert out_ap.ap[0][1] * out_ap.ap[1][1] == num_idxs
    assert in_ap.ap[-1][1] == out_ap.ap[-1][1] == elem_size
    assert in_ap.ap[0][0] == elem_step
    stride_bytes = elem_step * mybir.dt.size(in_ap.dtype)
    assert stride_bytes % 256 == 0
    stride_bytes_256 = stride_bytes // 256
    assert 0 < stride_bytes_256 < 256

    _in_ap = eng.lower_ap_dma(in_ap, for_custom_bir_dma=True)
    _idxs_ap = eng.lower_ap(idxs_ap)
    _out_ap = eng.lower_ap(out_ap)
    return eng.add_instruction(
        mybir.InstDMAGatherAnt(
            name=eng.bass.get_next_instruction_name(),
            ins=[*_in_ap, _idxs_ap,
                 eng.lower_val_access(eng.to_reg(num_idxs))],
            outs=[_out_ap],
            transpose=False,
            num_idxs=num_idxs,
            elem_size=elem_size,
            stride_bytes_256=stride_bytes_256,
            gen_mode=0,
            single_packet=True,
            queue_num=0,
            sbuf_tokens_per_rank=0,
            sbuf_free_dim_per_rank=0,
            sbuf_free_dim_pad_per_rank=0,
            sbuf_byte_offset=0,
        ))


# --------------------------------------------------------------------------
# device program
# --------------------------------------------------------------------------

def _build(meta):
    import concourse.bacc as bacc
    import concourse.mybir as mybir
    import concourse.tile as tile
    from concourse.masks import make_identity

    f32 = mybir.dt.float32
    AF = mybir.ActivationFunctionType
    OP = mybir.AluOpType

    S, B, SW, B0 = meta["S"], meta["B"], meta["SW"], meta["B0"]
    plane_off = meta["plane_off"]
    idx_woff, sidx_woff = meta["idx_woff"], meta["sidx_woff"]

    nc = bacc.Bacc()
    x_in = nc.declare_dram_parameter("x_sb", [P, NB * F1], f32, isOutput=False)
    deg_in = nc.declare_dram_parameter("deg", [P, NB], f32, isOutput=False)
    gidx_in = nc.declare_dram_parameter("gidx", [P, idx_woff[-1]],
                                        mybir.dt.int16, isOutput=False)
    sidx_in = nc.declare_dram_parameter("sidx", [P, sidx_woff[-1]],
                                        mybir.dt.int16, isOutput=False)
    wt_in = nc.declare_dram_parameter("wt", [64, 144], f32, isOutput=False)
    bt_in = nc.declare_dram_parameter("bt", [P, 144], f32, isOutput=False)
    out_ext = nc.declare_dram_parameter("out", [NLOC, F1], f32, isOutput=True)

    agin = nc.dram_tensor("agin", [NLOC, TF], f32)
    table = nc.dram_tensor("table", [NPAD, TF], f32, addr_space="Shared")
    aggd = nc.dram_tensor("aggd", [NLOC, TF], f32)

    rg = [list(range(N_CORES))]

    with tile.TileContext(nc) as tc:
        with (
            tc.tile_pool(name="persist", bufs=1) as pp,
            tc.tile_pool(name="slots", bufs=3) as sp,
            tc.tile_pool(name="tmp", bufs=4) as tp,
            tc.tile_pool(name="psum", bufs=2, space="PSUM") as psp,
        ):
            # ---- inputs ----
            x_t = pp.tile([P, NB * F1], f32)
            nc.sync.dma_start(out=x_t[:], in_=x_in[:, :])
            deg_t = pp.tile([P, NB], f32)
            nc.sync.dma_start(out=deg_t[:], in_=deg_in[:, :])
            gidx_t = pp.tile([P, idx_woff[-1]], mybir.dt.int16)
            nc.sync.dma_start(out=gidx_t[:], in_=gidx_in[:, :])
            sidx_t = pp.tile([P, sidx_woff[-1]], mybir.dt.int16)
            nc.sync.dma_start(out=sidx_t[:], in_=sidx_in[:, :])
            wt_t = pp.tile([64, 144], f32)
            nc.sync.dma_start(out=wt_t[:], in_=wt_in[:, :])
            bt_t = pp.tile([P, 144], f32)
            nc.sync.dma_start(out=bt_t[:], in_=bt_in[:, :])
            ident = pp.tile([P, P], f32)
            make_identity(nc, ident[:])

            # ---- dinv = (deg>0) / sqrt(max(deg,1)) ----
            dinv = pp.tile([P, NB], f32)
            msk = pp.tile([P, NB], f32)
            nc.vector.tensor_scalar(out=msk[:], in0=deg_t[:], scalar1=0.0,
                                    scalar2=None, op0=OP.is_gt)
            nc.vector.tensor_scalar(out=dinv[:], in0=deg_t[:], scalar1=1.0,
                                    scalar2=None, op0=OP.max)
            nc.scalar.sqrt(dinv[:], dinv[:])
            nc.vector.reciprocal(dinv[:], dinv[:])
            nc.vector.tensor_tensor(out=dinv[:], in0=dinv[:], in1=msk[:],
                                    op=OP.mult)

            shard = pp.tile([P, NB * TF], f32)
            nc.vector.memset(shard[:], 0.0)
            x1 = pp.tile([P, NB * 64], f32)
            aggw = pp.tile([P, NB * TF], f32)

            def build_table_l1():
                for b in range(NB):
                    nc.scalar.activation(
                        out=shard[:, b * TF:b * TF + F1],
                        in_=x_t[:, b * F1:(b + 1) * F1],
                        func=AF.Copy, scale=dinv[:, b:b + 1])

            def build_table_from(src_tile, wslice, fout):
                for b in range(NB):
                    tps = psp.tile([64, P], f32, tag="tp")
                    nc.tensor.transpose(out=tps[:],
                                        in_=src_tile[:, b * 64:(b + 1) * 64],
                                        identity=ident[:])
                    lhsT = tp.tile([64, P], f32, tag="lhsT")
                    nc.scalar.copy(lhsT[:], tps[:])
                    mm = psp.tile([P, 64], f32, tag="mm")
                    nc.tensor.matmul(out=mm[:, :fout], lhsT=lhsT[:],
                                     rhs=wt_t[:, wslice],
                                     start=True, stop=True)
                    nc.scalar.activation(out=shard[:, b * TF:b * TF + fout],
                                         in_=mm[:, :fout], func=AF.Copy,
                                         scale=dinv[:, b:b + 1])

            def aggregate(fl):
                nc.sync.dma_start(
                    out=agin[:, :].rearrange("(p b) f -> p b f", p=P),
                    in_=shard[:].rearrange("p (b f) -> p b f", b=NB))
                nc.gpsimd.collective_compute(
                    "AllGather", OP.bypass, replica_groups=rg,
                    ins=[agin[:, :]], outs=[table[:, :]])
                for w in range(N_WIN):
                    base, size = W_BASES[w], W_SIZES[w]
                    if w == 0:
                        nc.vector.memset(aggw[:, :NB * fl], 0.0)
                        # zero the DRAM accumulator from the zeroed tile
                        nc.sync.dma_start(
                            out=aggd[:, 0:fl].rearrange("(p b) f -> p b f", p=P),
                            in_=aggw[:, :NB * fl].rearrange("p (b f) -> p b f",
                                                            b=NB))
                    else:
                        nc.vector.memset(aggw[:, :B0[w] * fl], 0.0)
                    c0 = 0
                    while c0 < SW[w]:
                        c1 = min(c0 + GCHUNK, SW[w])
                        ncols = c1 - c0
                        st = sp.tile([P, GCHUNK * 64], f32, tag="slot")
                        _dma_gather_raw(
                            nc.gpsimd,
                            out_ap=st[:, :ncols * fl].rearrange(
                                "p (c f) -> p c f", c=ncols),
                            in_ap=table[base:base + size, 0:fl],
                            idxs_ap=gidx_t[:, idx_woff[w] + c0 * 8:
                                           idx_woff[w] + c1 * 8],
                            num_idxs=ncols * P,
                            elem_size=fl, elem_step=TF)
                        for k in range(len(B[w])):
                            s0 = max(c0, int(plane_off[w][k]))
                            s1 = min(c1, int(plane_off[w][k + 1]))
                            if s0 >= s1:
                                continue
                            bs = s0 - int(plane_off[w][k])
                            be = s1 - int(plane_off[w][k])
                            nc.vector.tensor_tensor(
                                out=aggw[:, bs * fl:be * fl],
                                in0=aggw[:, bs * fl:be * fl],
                                in1=st[:, (s0 - c0) * fl:(s1 - c0) * fl],
                                op=OP.add)
                        c0 = c1
                    sb0 = 0
                    while sb0 < B0[w]:
                        nb_ = min(SCHUNK, B0[w] - sb0)
                        nc.gpsimd.dma_scatter_add(
                            out_ap=aggd[:, 0:fl],
                            in_ap=aggw[:, sb0 * fl:(sb0 + nb_) * fl].rearrange(
                                "p (b f) -> p b f", b=nb_),
                            idxs_ap=sidx_t[:, sidx_woff[w] + sb0 * 8:
                                           sidx_woff[w] + (sb0 + nb_) * 8],
                            num_idxs=nb_ * P, num_idxs_reg=nb_ * P,
                            elem_size=fl, elem_step=TF)
                        sb0 += nb_
                agg_sb = aggw  # reuse for canonical readback
                nc.sync.dma_start(
                    out=agg_sb[:, :NB * fl].rearrange("p (b f) -> p b f", b=NB),
                    in_=aggd[:, 0:fl].rearrange("(p b) f -> p b f", p=P))
                return agg_sb

            # =============== conv1 ===============
            build_table_l1()
            agg1 = aggregate(F1)
            for b in range(NB):
                t1 = tp.tile([P, F1], f32, tag="t1")
                nc.scalar.activation(out=t1[:], in_=agg1[:, b * F1:(b + 1) * F1],
                                     func=AF.Copy, scale=dinv[:, b:b + 1])
                tps = psp.tile([F1, P], f32, tag="tp1")
                nc.tensor.transpose(out=tps[:], in_=t1[:], identity=ident[:])
                lhsT = tp.tile([F1, P], f32, tag="lhsT1")
                nc.scalar.copy(lhsT[:], tps[:])
                mm = psp.tile([P, 64], f32, tag="mm")
                nc.tensor.matmul(out=mm[:], lhsT=lhsT[:16, :],
                                 rhs=wt_t[:16, 0:64], start=True, stop=True)
                nc.vector.tensor_tensor(out=x1[:, b * 64:(b + 1) * 64],
                                        in0=mm[:], in1=bt_t[:, 0:64], op=OP.add)
                nc.vector.tensor_scalar(out=x1[:, b * 64:(b + 1) * 64],
                                        in0=x1[:, b * 64:(b + 1) * 64],
                                        scalar1=0.0, scalar2=None, op0=OP.max)

            # =============== conv2 ===============
            build_table_from(x1, slice(64, 128), 64)
            agg2 = aggregate(64)
            for b in range(NB):
                yb = tp.tile([P, 64], f32, tag="yb")
                nc.scalar.activation(out=yb[:], in_=agg2[:, b * 64:(b + 1) * 64],
                                     func=AF.Copy, scale=dinv[:, b:b + 1])
                nc.vector.tensor_tensor(out=yb[:], in0=yb[:],
                                        in1=bt_t[:, 64:128], op=OP.add)
                nc.vector.tensor_scalar(out=yb[:], in0=yb[:], scalar1=0.0,
                                        scalar2=None, op0=OP.max)
                nc.vector.tensor_tensor(out=x1[:, b * 64:(b + 1) * 64],
                                        in0=x1[:, b * 64:(b + 1) * 64],
                                        in1=yb[:], op=OP.add)

            # =============== conv3 ===============
            build_table_from(x1, slice(128, 144), F1)
            agg3 = aggregate(F1)
            u = shard  # reuse
            for b in range(NB):
                nc.scalar.activation(out=u[:, b * F1:(b + 1) * F1],
                                     in_=agg3[:, b * F1:(b + 1) * F1],
                                     func=AF.Copy, scale=dinv[:, b:b + 1])
                nc.vector.tensor_tensor(out=u[:, b * F1:(b + 1) * F1],
                                        in0=u[:, b * F1:(b + 1) * F1],
                                        in1=bt_t[:, 128:144], op=OP.add)
            mx = pp.tile([P, NB], f32)
            nc.vector.tensor_reduce(out=mx[:],
                                    in_=u[:, :NB * F1].rearrange(
                                        "p (b f) -> p b f", b=NB),
                                    axis=mybir.AxisListType.X, op=OP.max)
            nc.vector.tensor_scalar(out=mx[:], in0=mx[:], scalar1=-1.0,
                                    scalar2=None, op0=OP.mult)
            et = x1  # reuse
            for b in range(NB):
                nc.scalar.activation(out=et[:, b * F1:(b + 1) * F1],
                                     in_=u[:, b * F1:(b + 1) * F1],
                                     func=AF.Exp, bias=mx[:, b:b + 1])
            sm = pp.tile([P, NB], f32)
            nc.vector.tensor_reduce(out=sm[:],
                                    in_=et[:, :NB * F1].rearrange(
                                        "p (b f) -> p b f", b=NB),
                                    axis=mybir.AxisListType.X, op=OP.add)
            lg = msk  # reuse
            nc.scalar.activation(out=lg[:], in_=sm[:], func=AF.Ln)
            nc.vector.tensor_tensor(out=lg[:], in0=mx[:], in1=lg[:],
                                    op=OP.subtract)
            for b in range(NB):
                nc.vector.tensor_scalar(out=u[:, b * F1:(b + 1) * F1],
                                        in0=u[:, b * F1:(b + 1) * F1],
                                        scalar1=lg[:, b:b + 1], scalar2=None,
                                        op0=OP.add)
            nc.sync.dma_start(
                out=out_ext[:, :].rearrange("(p b) f -> p b f", p=P),
                in_=u[:, :NB * F1].rearrange("p (b f) -> p b f", b=NB))

    nc.finalize()
    return nc


# --------------------------------------------------------------------------

def kernel(**inputs):
    x = np.asarray(inputs["x"], dtype=np.float32)
    edge_index = np.asarray(inputs["edge_index"])
    W1 = np.asarray(inputs["W1"], dtype=np.float32)
    b1 = np.asarray(inputs["b1"], dtype=np.float32)
    W2 = np.asarray(inputs["W2"], dtype=np.float32)
    b2 = np.asarray(inputs["b2"], dtype=np.float32)
    W3 = np.asarray(inputs["W3"], dtype=np.float32)
    b3 = np.asarray(inputs["b3"], dtype=np.float32)

    meta, perm, idx_cat, sidx_cat = _prep(edge_index)
    in_maps = _pack_inputs(x, W1, b1, W2, b2, W3, b3, perm, idx_cat, sidx_cat)
    nc = _build(meta)

    from concourse.bass_utils import run_bass_kernel_spmd
    trace = os.environ.get("GCN_TRACE") == "1"
    res = run_bass_kernel_spmd(nc, in_maps, core_ids=list(range(N_CORES)),
                               trace=trace)
    kernel.last_result = res
    kernel.last_nc = nc
    kernel.last_in_maps = in_maps

    out = np.empty((N_NODES, 10), dtype=np.float32)
    core_of, r_loc = perm["core_of"], perm["r_loc"]
    for c in range(N_CORES):
        oc = res.results[c]["out"]
        ids = np.flatnonzero(core_of == c)
        out[ids] = oc[r_loc[ids], :10]
    return out



# revision 3
# speedup vs baseline: 7.4908x; 7.4908x over previous
"""3-layer GCN (gcn_norm + 3x gcn_conv + log_softmax) on 8 TRN2 NeuronCores.

Strategy (dst-sharded, graph-parallel):
  - Nodes split contiguously across 8 cores (12500 each, padded to
    12544 = 128*98 rows). The kernel works in a permuted "table row"
    space; the host un-permutes at the end.
  - GCN algebra refactored so no per-edge norm is needed:
        conv(h) = D^-1/2 (A+I) D^-1/2 (h W) + b
    with g = (h W) * dinv:  out[n] = dinv[n] * (g[n] + sum_{e:dst=n} g[src_e]) + b
    so each conv is: build table g (pre-scaled rows), AllGather, gather+sum
    rows per destination, scale by dinv, add bias.
  - gather+sum: bulk dma_gather (SWDGE custom op) of table rows into SBUF
    slot tiles, unit-stride DVE plane adds, then dma_scatter_add into a
    DRAM accumulator to reconcile per-window node orders (int16 gather
    indices only span a 32768-row window -> edges split into 4 source
    windows, each with its own degree-sorted block structure so slot
    padding stays small).
  - The tiny weight matmuls (13->64->64->10) run per 128-node block on
    the TensorEngine (transpose-via-identity + matmul).

kernel(**inputs) takes FULL unsharded inputs, returns FULL [100000, 10]
float32 log-softmax output.
"""
import os
import sys

import numpy as np

if "/opt/trn_rl_repo" not in sys.path:
    sys.path.insert(0, "/opt/trn_rl_repo")

N_NODES = 100000
N_EDGES = 1600000
N_CORES = 8
P = 128
NB = 98                      # node blocks per core
NLOC = P * NB                # 12544 padded nodes per core
NPC = N_NODES // N_CORES     # 12500 real nodes per core
NPAD = N_CORES * NLOC        # 100352 table rows
WIN = 32768                  # int16 index window (rows)
N_WIN = 4
W_BASES = [0, WIN, 2 * WIN, 3 * WIN]
W_SIZES = [WIN, WIN, WIN, NPAD - 3 * WIN]
TF = 64                      # table row stride in f32 (256B)
F1 = 16                      # conv1/conv3 gathered row width (f32)
GCHUNK = int(os.environ.get("GCN_GCHUNK", "8"))   # slot cols per dma_gather
SCHUNK = int(os.environ.get("GCN_SCHUNK", "24"))  # blocks per dma_scatter_add
# timing-only ablations (produce WRONG results; never set when grading)
ABL_NO_CC = os.environ.get("GCN_NO_CC") == "1"
ABL_NO_GATHER = os.environ.get("GCN_NO_GATHER") == "1"
ABL_NO_SCATTER = os.environ.get("GCN_NO_SCATTER") == "1"


# --------------------------------------------------------------------------
# host-side graph preprocessing (integer/layout work only)
# --------------------------------------------------------------------------

def _prep(edge_index):
    src = np.asarray(edge_index[0], dtype=np.int64)
    dst = np.asarray(edge_index[1], dtype=np.int64)

    deg_in = np.bincount(dst, minlength=N_NODES)
    core_of = np.arange(N_NODES) // NPC

    # canonical within-core order: total degree descending
    rank = np.empty(N_NODES, dtype=np.int64)
    for c in range(N_CORES):
        ids = np.arange(c * NPC, (c + 1) * NPC)
        o = np.argsort(-deg_in[ids], kind="stable")
        rank[ids[o]] = np.arange(NPC)
    p_of = rank % P
    b_of = rank // P
    r_loc = p_of * NB + b_of                 # canonical local row
    g_row = core_of * NLOC + r_loc           # global table row

    # message entries = edges + self loops; window = src row // WIN
    all_dst = np.concatenate([dst, np.arange(N_NODES)])
    all_gsrc = np.concatenate([g_row[src], g_row])
    all_w = all_gsrc // WIN
    all_core = all_dst // NPC

    # per (node, window) counts
    wcnt = np.bincount(all_dst * N_WIN + all_w,
                       minlength=N_NODES * N_WIN).reshape(N_NODES, N_WIN)

    # per (core, window): window-degree-sorted ranks
    wrank = np.empty((N_NODES, N_WIN), dtype=np.int64)
    wdeg_sorted = np.zeros((N_CORES, N_WIN, NLOC), dtype=np.int64)
    wnode_rloc = np.zeros((N_CORES, N_WIN, NLOC), dtype=np.int64)
    dr = np.arange(NPC, NLOC)
    dummy_rloc = (dr % P) * NB + dr // P
    for c in range(N_CORES):
        ids = np.arange(c * NPC, (c + 1) * NPC)
        for w in range(N_WIN):
            o = np.argsort(-wcnt[ids, w], kind="stable")
            wrank[ids[o], w] = np.arange(NPC)
            wdeg_sorted[c, w, :NPC] = wcnt[ids[o], w]
            wnode_rloc[c, w, :NPC] = r_loc[ids[o]]
            wnode_rloc[c, w, NPC:] = dummy_rloc

    # shared slot structure (cross-core max; non-increasing in b)
    S = [wdeg_sorted[:, w, ::P].max(axis=0).astype(np.int64)
         for w in range(N_WIN)]
    B = []
    for w in range(N_WIN):
        s0 = int(S[w][0])
        B.append([int((S[w] > k).sum()) for k in range(s0)])
    SW = [int(S[w].sum()) for w in range(N_WIN)]
    plane_off = [np.concatenate([[0], np.cumsum(B[w])]).astype(np.int64)
                 for w in range(N_WIN)]

    # guaranteed-zero padding row per window (a dummy node; dinv = 0)
    all_dummies = (np.arange(N_CORES)[:, None] * NLOC +
                   dummy_rloc[None, :]).ravel()
    zero_row = []
    for w in range(N_WIN):
        base, size = W_BASES[w], W_SIZES[w]
        cand = all_dummies[(all_dummies >= base) & (all_dummies < base + size)]
        assert len(cand) > 0, f"no zero row in window {w}"
        zero_row.append(int(cand[0]))

    # fill gather index arrays (slot col of k-th entry = plane_off[k] + b)
    idx_lin = [np.full((N_CORES, SW[w] * P), zero_row[w] - W_BASES[w],
                       dtype=np.int64) for w in range(N_WIN)]
    ekey = (all_core * N_WIN + all_w) * NLOC + wrank[all_dst, all_w]
    order = np.argsort(ekey, kind="stable")
    se_key = ekey[order]
    se_gsrc = all_gsrc[order]
    is_start = np.ones(len(se_key), dtype=bool)
    is_start[1:] = se_key[1:] != se_key[:-1]
    grp_start_pos = np.flatnonzero(is_start)
    grp_id = np.cumsum(is_start) - 1
    k_within = np.arange(len(se_key)) - grp_start_pos[grp_id]

    se_c = se_key // (N_WIN * NLOC)
    se_w = (se_key // NLOC) % N_WIN
    se_r = se_key % NLOC
    se_p = se_r % P
    se_b = se_r // P
    for w in range(N_WIN):
        m = se_w == w
        col = plane_off[w][k_within[m]] + se_b[m]
        idx_lin[w][se_c[m], col * P + se_p[m]] = se_gsrc[m] - W_BASES[w]
        assert idx_lin[w].min() >= 0 and idx_lin[w].max() < W_SIZES[w]

    def wrap16(lin2d):
        # [cores, L] -> [cores, 16, L//16] with linear[i] at [:, i%16, i//16],
        # then replicate the 16-row pattern across 128 partitions.
        n = lin2d.shape[1]
        a = lin2d.reshape(N_CORES, n // 16, 16).transpose(0, 2, 1)
        a = np.ascontiguousarray(a).astype(np.int16)
        return np.tile(a, (1, 8, 1))  # [cores, 128, n//16]

    idx_cat = np.concatenate([wrap16(idx_lin[w]) for w in range(N_WIN)], axis=2)
    idx_woff = np.concatenate(
        [[0], np.cumsum([SW[w] * 8 for w in range(N_WIN)])]).astype(int)

    # scatter indices: token i (= w-rank) -> canonical local row
    B0 = [B[w][0] for w in range(N_WIN)]
    sidx_cat = np.concatenate(
        [wrap16(wnode_rloc[:, w, :B0[w] * P]) for w in range(N_WIN)], axis=2)
    sidx_woff = np.concatenate(
        [[0], np.cumsum([B0[w] * 8 for w in range(N_WIN)])]).astype(int)

    meta = dict(S=[s.tolist() for s in S], B=B, SW=SW, B0=B0,
                plane_off=[p.tolist() for p in plane_off],
                idx_woff=idx_woff.tolist(), sidx_woff=sidx_woff.tolist())
    perm = dict(core_of=core_of, r_loc=r_loc, deg_in=deg_in,
                p_of=p_of, b_of=b_of)
    return meta, perm, idx_cat, sidx_cat


def _pack_inputs(x, W1, b1, W2, b2, W3, b3, perm, idx_cat, sidx_cat):
    core_of, r_loc = perm["core_of"], perm["r_loc"]
    p_of, b_of, deg_in = perm["p_of"], perm["b_of"], perm["deg_in"]

    x_sb = np.zeros((N_CORES, P, NB * F1), dtype=np.float32)
    for j in range(13):
        x_sb[core_of, p_of, b_of * F1 + j] = x[:, j]
    deg_f = np.zeros((N_CORES, P, NB), dtype=np.float32)
    deg_f[core_of, p_of, b_of] = (deg_in + 1).astype(np.float32)

    wt = np.zeros((64, 144), dtype=np.float32)
    wt[:13, 0:64] = W1
    wt[:, 64:128] = W2
    wt[:, 128:138] = W3
    bt = np.zeros((P, 144), dtype=np.float32)
    bt[:, 0:64] = b1[None, :]
    bt[:, 64:128] = b2[None, :]
    bt[:, 128:138] = b3[None, :]
    bt[:, 138:144] = -1e30  # pad classes for batched log_softmax

    return [{
        "x_sb": x_sb[c], "deg": deg_f[c],
        "gidx": idx_cat[c], "sidx": sidx_cat[c],
        "wt": wt, "bt": bt,
    } for c in range(N_CORES)]


# --------------------------------------------------------------------------
# raw dma_gather emitter: bass.BassGpSimd.dma_gather minus the
# transpose-only elem%256B restriction (the q7 ucode only requires the row
# *stride* to be a multiple of 256B for the non-transpose HBM path).
# --------------------------------------------------------------------------

def _dma_gather_raw(eng, out_ap, in_ap, idxs_ap, num_idxs, elem_size,
                    elem_step):
    import concourse.mybir as mybir
    from concourse import ap_utils
    from concourse.bass import MemorySpace

    assert idxs_ap.dtype == mybir.dt.int16
    assert in_ap.dtype == out_ap.dtype
    assert in_ap.space == MemorySpace.DRAM
    assert idxs_ap.space == MemorySpace.SBUF
    assert out_ap.space == MemorySpace.SBUF
    assert ap_utils.ap_is_contiguous(out_ap.ap[1:])
    assert ap_utils.ap_is_contiguous(idxs_ap.ap[1:])
    assert num_idxs % P == 0
    assert out_ap.ap[0][1] * out_ap.ap[1][1] == num_idxs
    assert in_ap.ap[-1][1] == out_ap.ap[-1][1] == elem_size
    assert in_ap.ap[0][0] == elem_step
    stride_bytes = elem_step * mybir.dt.size(in_ap.dtype)
    assert stride_bytes % 256 == 0
    stride_bytes_256 = stride_bytes // 256
    assert 0 < stride_bytes_256 < 256

    _in_ap = eng.lower_ap_dma(in_ap, for_custom_bir_dma=True)
    _idxs_ap = eng.lower_ap(idxs_ap)
    _out_ap = eng.lower_ap(out_ap)
    return eng.add_instruction(
        mybir.InstDMAGatherAnt(
            name=eng.bass.get_next_instruction_name(),
            ins=[*_in_ap, _idxs_ap,
                 eng.lower_val_access(eng.to_reg(num_idxs))],
            outs=[_out_ap],
            transpose=False,
            num_idxs=num_idxs,
            elem_size=elem_size,
            stride_bytes_256=stride_bytes_256,
            gen_mode=0,
            single_packet=True,
            queue_num=0,
            sbuf_tokens_per_rank=0,
            sbuf_free_dim_per_rank=0,
            sbuf_free_dim_pad_per_rank=0,
            sbuf_byte_offset=0,
        ))


# --------------------------------------------------------------------------
# device program
# --------------------------------------------------------------------------

def _build(meta):
    import concourse.bacc as bacc
    import concourse.mybir as mybir
    import concourse.tile as tile
    from concourse.masks import make_identity

    f32 = mybir.dt.float32
    AF = mybir.ActivationFunctionType
    OP = mybir.AluOpType

    S, B, SW, B0 = meta["S"], meta["B"], meta["SW"], meta["B0"]
    plane_off = meta["plane_off"]
    idx_woff, sidx_woff = meta["idx_woff"], meta["sidx_woff"]

    nc = bacc.Bacc()
    x_in = nc.declare_dram_parameter("x_sb", [P, NB * F1], f32, isOutput=False)
    deg_in = nc.declare_dram_parameter("deg", [P, NB], f32, isOutput=False)
    gidx_in = nc.declare_dram_parameter("gidx", [P, idx_woff[-1]],
                                        mybir.dt.int16, isOutput=False)
    sidx_in = nc.declare_dram_parameter("sidx", [P, sidx_woff[-1]],
                                        mybir.dt.int16, isOutput=False)
    wt_in = nc.declare_dram_parameter("wt", [64, 144], f32, isOutput=False)
    bt_in = nc.declare_dram_parameter("bt", [P, 144], f32, isOutput=False)
    out_ext = nc.declare_dram_parameter("out", [NLOC, F1], f32, isOutput=True)

    agin = nc.dram_tensor("agin", [NLOC, TF], f32)
    table = nc.dram_tensor("table", [NPAD, TF], f32, addr_space="Shared")
    aggd = nc.dram_tensor("aggd", [NLOC, TF], f32)

    rg = [list(range(N_CORES))]

    with tile.TileContext(nc) as tc:
        with (
            tc.tile_pool(name="persist", bufs=1) as pp,
            tc.tile_pool(name="slots", bufs=3) as sp,
            tc.tile_pool(name="tmp", bufs=4) as tp,
            tc.tile_pool(name="psum", bufs=2, space="PSUM") as psp,
        ):
            # ---- inputs ----
            x_t = pp.tile([P, NB * F1], f32)
            nc.sync.dma_start(out=x_t[:], in_=x_in[:, :])
            deg_t = pp.tile([P, NB], f32)
            nc.sync.dma_start(out=deg_t[:], in_=deg_in[:, :])
            gidx_t = pp.tile([P, idx_woff[-1]], mybir.dt.int16)
            nc.sync.dma_start(out=gidx_t[:], in_=gidx_in[:, :])
            sidx_t = pp.tile([P, sidx_woff[-1]], mybir.dt.int16)
            nc.sync.dma_start(out=sidx_t[:], in_=sidx_in[:, :])
            wt_t = pp.tile([64, 144], f32)
            nc.sync.dma_start(out=wt_t[:], in_=wt_in[:, :])
            bt_t = pp.tile([P, 144], f32)
            nc.sync.dma_start(out=bt_t[:], in_=bt_in[:, :])
            ident = pp.tile([P, P], f32)
            make_identity(nc, ident[:])

            # ---- dinv = (deg>0) / sqrt(max(deg,1)) ----
            dinv = pp.tile([P, NB], f32)
            msk = pp.tile([P, NB], f32)
            nc.vector.tensor_scalar(out=msk[:], in0=deg_t[:], scalar1=0.0,
                                    scalar2=None, op0=OP.is_gt)
            nc.vector.tensor_scalar(out=dinv[:], in0=deg_t[:], scalar1=1.0,
                                    scalar2=None, op0=OP.max)
            nc.scalar.sqrt(dinv[:], dinv[:])
            nc.vector.reciprocal(dinv[:], dinv[:])
            nc.vector.tensor_tensor(out=dinv[:], in0=dinv[:], in1=msk[:],
                                    op=OP.mult)

            shard = pp.tile([P, NB * TF], f32)
            nc.vector.memset(shard[:], 0.0)
            x1 = pp.tile([P, NB * 64], f32)
            aggw = pp.tile([P, NB * TF], f32)

            def build_table_l1():
                for b in range(NB):
                    nc.scalar.activation(
                        out=shard[:, b * TF:b * TF + F1],
                        in_=x_t[:, b * F1:(b + 1) * F1],
                        func=AF.Copy, scale=dinv[:, b:b + 1])

            def build_table_from(src_tile, wslice, fout):
                for b in range(NB):
                    tps = psp.tile([64, P], f32, tag="tp")
                    nc.tensor.transpose(out=tps[:],
                                        in_=src_tile[:, b * 64:(b + 1) * 64],
                                        identity=ident[:])
                    lhsT = tp.tile([64, P], f32, tag="lhsT")
                    nc.scalar.copy(lhsT[:], tps[:])
                    mm = psp.tile([P, 64], f32, tag="mm")
                    nc.tensor.matmul(out=mm[:, :fout], lhsT=lhsT[:],
                                     rhs=wt_t[:, wslice],
                                     start=True, stop=True)
                    nc.scalar.activation(out=shard[:, b * TF:b * TF + fout],
                                         in_=mm[:, :fout], func=AF.Copy,
                                         scale=dinv[:, b:b + 1])

            def aggregate(fl):
                nc.sync.dma_start(
                    out=agin[:, :].rearrange("(p b) f -> p b f", p=P),
                    in_=shard[:].rearrange("p (b f) -> p b f", b=NB))
                nc.gpsimd.collective_compute(
                    "AllGather", OP.bypass, replica_groups=rg,
                    ins=[agin[:, :]], outs=[table[:, :]])
                for w in range(N_WIN):
                    base, size = W_BASES[w], W_SIZES[w]
                    if w == 0:
                        nc.vector.memset(aggw[:, :NB * fl], 0.0)
                        # zero the DRAM accumulator from the zeroed tile
                        nc.sync.dma_start(
                            out=aggd[:, 0:fl].rearrange("(p b) f -> p b f", p=P),
                            in_=aggw[:, :NB * fl].rearrange("p (b f) -> p b f",
                                                            b=NB))
                    else:
                        nc.vector.memset(aggw[:, :B0[w] * fl], 0.0)
                    c0 = 0
                    while c0 < SW[w]:
                        c1 = min(c0 + GCHUNK, SW[w])
                        ncols = c1 - c0
                        st = sp.tile([P, GCHUNK * 64], f32, tag="slot")
                        _dma_gather_raw(
                            nc.gpsimd,
                            out_ap=st[:, :ncols * fl].rearrange(
                                "p (c f) -> p c f", c=ncols),
                            in_ap=table[base:base + size, 0:fl],
                            idxs_ap=gidx_t[:, idx_woff[w] + c0 * 8:
                                           idx_woff[w] + c1 * 8],
                            num_idxs=ncols * P,
                            elem_size=fl, elem_step=TF)
                        for k in range(len(B[w])):
                            s0 = max(c0, int(plane_off[w][k]))
                            s1 = min(c1, int(plane_off[w][k + 1]))
                            if s0 >= s1:
                                continue
                            bs = s0 - int(plane_off[w][k])
                            be = s1 - int(plane_off[w][k])
                            nc.vector.tensor_tensor(
                                out=aggw[:, bs * fl:be * fl],
                                in0=aggw[:, bs * fl:be * fl],
                                in1=st[:, (s0 - c0) * fl:(s1 - c0) * fl],
                                op=OP.add)
                        c0 = c1
                    sb0 = 0
                    while sb0 < B0[w]:
                        nb_ = min(SCHUNK, B0[w] - sb0)
                        nc.gpsimd.dma_scatter_add(
                            out_ap=aggd[:, 0:fl],
                            in_ap=aggw[:, sb0 * fl:(sb0 + nb_) * fl].rearrange(
                                "p (b f) -> p b f", b=nb_),
                            idxs_ap=sidx_t[:, sidx_woff[w] + sb0 * 8:
                                           sidx_woff[w] + (sb0 + nb_) * 8],
                            num_idxs=nb_ * P, num_idxs_reg=nb_ * P,
                            elem_size=fl, elem_step=TF)
                        sb0 += nb_
                agg_sb = aggw  # reuse for canonical readback
                nc.sync.dma_start(
                    out=agg_sb[:, :NB * fl].rearrange("p (b f) -> p b f", b=NB),
                    in_=aggd[:, 0:fl].rearrange("(p b) f -> p b f", p=P))
                return agg_sb

            # =============== conv1 ===============
            build_table_l1()
            agg1 = aggregate(F1)
            for b in range(NB):
                t1 = tp.tile([P, F1], f32, tag="t1")
                nc.scalar.activation(out=t1[:], in_=agg1[:, b * F1:(b + 1) * F1],
                                     func=AF.Copy, scale=dinv[:, b:b + 1])
                tps = psp.tile([F1, P], f32, tag="tp1")
                nc.tensor.transpose(out=tps[:], in_=t1[:], identity=ident[:])
                lhsT = tp.tile([F1, P], f32, tag="lhsT1")
                nc.scalar.copy(lhsT[:], tps[:])
                mm = psp.tile([P, 64], f32, tag="mm")
                nc.tensor.matmul(out=mm[:], lhsT=lhsT[:16, :],
                                 rhs=wt_t[:16, 0:64], start=True, stop=True)
                nc.vector.tensor_tensor(out=x1[:, b * 64:(b + 1) * 64],
                                        in0=mm[:], in1=bt_t[:, 0:64], op=OP.add)
                nc.vector.tensor_scalar(out=x1[:, b * 64:(b + 1) * 64],
                                        in0=x1[:, b * 64:(b + 1) * 64],
                                        scalar1=0.0, scalar2=None, op0=OP.max)

            # =============== conv2 ===============
            build_table_from(x1, slice(64, 128), 64)
            agg2 = aggregate(64)
            for b in range(NB):
                yb = tp.tile([P, 64], f32, tag="yb")
                nc.scalar.activation(out=yb[:], in_=agg2[:, b * 64:(b + 1) * 64],
                                     func=AF.Copy, scale=dinv[:, b:b + 1])
                nc.vector.tensor_tensor(out=yb[:], in0=yb[:],
                                        in1=bt_t[:, 64:128], op=OP.add)
                nc.vector.tensor_scalar(out=yb[:], in0=yb[:], scalar1=0.0,
                                        scalar2=None, op0=OP.max)
                nc.vector.tensor_tensor(out=x1[:, b * 64:(b + 1) * 64],
                                        in0=x1[:, b * 64:(b + 1) * 64],
                                        in1=yb[:], op=OP.add)

            # =============== conv3 ===============
            build_table_from(x1, slice(128, 144), F1)
            agg3 = aggregate(F1)
            u = shard  # reuse
            for b in range(NB):
                nc.scalar.activation(out=u[:, b * F1:(b + 1) * F1],
                                     in_=agg3[:, b * F1:(b + 1) * F1],
                                     func=AF.Copy, scale=dinv[:, b:b + 1])
                nc.vector.tensor_tensor(out=u[:, b * F1:(b + 1) * F1],
                                        in0=u[:, b * F1:(b + 1) * F1],
                                        in1=bt_t[:, 128:144], op=OP.add)
            mx = pp.tile([P, NB], f32)
            nc.vector.tensor_reduce(out=mx[:],
                                    in_=u[:, :NB * F1].rearrange(
                                        "p (b f) -> p b f", b=NB),
                                    axis=mybir.AxisListType.X, op=OP.max)
            nc.vector.tensor_scalar(out=mx[:], in0=mx[:], scalar1=-1.0,
                                    scalar2=None, op0=OP.mult)
            et = x1  # reuse
            for b in range(NB):
                nc.scalar.activation(out=et[:, b * F1:(b + 1) * F1],
                                     in_=u[:, b * F1:(b + 1) * F1],
                                     func=AF.Exp, bias=mx[:, b:b + 1])
            sm = pp.tile([P, NB], f32)
            nc.vector.tensor_reduce(out=sm[:],
                                    in_=et[:, :NB * F1].rearrange(
                                        "p (b f) -> p b f", b=NB),
                                    axis=mybir.AxisListType.X, op=OP.add)
            lg = msk  # reuse
            nc.scalar.activation(out=lg[:], in_=sm[:], func=AF.Ln)
            nc.vector.tensor_tensor(out=lg[:], in0=mx[:], in1=lg[:],
                                    op=OP.subtract)
            for b in range(NB):
                nc.vector.tensor_scalar(out=u[:, b * F1:(b + 1) * F1],
                                        in0=u[:, b * F1:(b + 1) * F1],
                                        scalar1=lg[:, b:b + 1], scalar2=None,
                                        op0=OP.add)
            nc.sync.dma_start(
                out=out_ext[:, :].rearrange("(p b) f -> p b f", p=P),
                in_=u[:, :NB * F1].rearrange("p (b f) -> p b f", b=NB))

    nc.finalize()
    return nc


# --------------------------------------------------------------------------

def kernel(**inputs):
    x = np.asarray(inputs["x"], dtype=np.float32)
    edge_index = np.asarray(inputs["edge_index"])
    W1 = np.asarray(inputs["W1"], dtype=np.float32)
    b1 = np.asarray(inputs["b1"], dtype=np.float32)
    W2 = np.asarray(inputs["W2"], dtype=np.float32)
    b2 = np.asarray(inputs["b2"], dtype=np.float32)
    W3 = np.asarray(inputs["W3"], dtype=np.float32)
    b3 = np.asarray(inputs["b3"], dtype=np.float32)

    meta, perm, idx_cat, sidx_cat = _prep(edge_index)
    in_maps = _pack_inputs(x, W1, b1, W2, b2, W3, b3, perm, idx_cat, sidx_cat)
    nc = _build(meta)

    from concourse.bass_utils import run_bass_kernel_spmd
    trace = os.environ.get("GCN_TRACE") == "1"
    res = run_bass_kernel_spmd(nc, in_maps, core_ids=list(range(N_CORES)),
                               trace=trace)
    kernel.last_result = res
    kernel.last_nc = nc
    kernel.last_in_maps = in_maps

    out = np.empty((N_NODES, 10), dtype=np.float32)
    core_of, r_loc = perm["core_of"], perm["r_loc"]
    for c in range(N_CORES):
        oc = res.results[c]["out"]
        ids = np.flatnonzero(core_of == c)
        out[ids] = oc[r_loc[ids], :10]
    return out

